# Initial kernel scaffold
#
"""GNN message passing on 8 Trainium2 NeuronCores.

Reference computation:
    h = x @ W                       # [N, D]
    msg = h[src]                    # [E, D]
    out = relu(segment_sum(msg, dst, N))

Key identity used here: segment_sum(x[src] @ W) == segment_sum(x[src]) @ W,
so we aggregate raw x rows and apply the small 128x128 matmul once per
output tile at the end.

Distribution: shard edges by DESTINATION range (6272 nodes per core).
Outputs are disjoint -> no all-reduce. Each core:
  1. dma_gather's x rows for its edges (from a replicated node table in
     its local HBM; the table is split into "lo"/"hi" halves so gather
     indices fit in int16, with a zeros row at index 0 for padding),
  2. aggregates them into per-128-dst-window PSUM accumulators via
     TensorE matmuls against one-hot selection matrices built on-device
     with a single DVE compare (iota == dst_local) per 128-edge tile,
  3. applies the final @W matmul + relu per window and DMAs out.

Host-side packing sorts edges into a fixed (window, src-half) slot grid
so all 8 cores run one identical SPMD program.
"""

import numpy as np

P = 128
D = 128
N_NODES = 50000
N_CORES = 8
NW = 49                 # 128-dst windows per core
DPC = NW * P            # dsts per core = 6272
CG = 7                  # windows per gather chunk (NW % CG == 0)
LO_MAX = 32767          # src < LO_MAX -> lo table (idx = src+1 <= 32767)
LO_ROWS = LO_MAX + 1
HI_ROWS = N_NODES - LO_MAX + 1

_NC_CACHE = {}


def _build_nc(t_lo, t_hi, nw=NW, cg=CG, lo_rows=LO_ROWS, hi_rows=HI_ROWS,
              bench_reps=1, parts="all", dma_scratch=16384):
    key = (t_lo, t_hi, nw, cg, lo_rows, hi_rows, bench_reps, parts, dma_scratch)
    if key in _NC_CACHE:
        return _NC_CACHE[key]

    import concourse.bacc as bacc
    import concourse.mybir as mybir
    import concourse.tile as tile
    from concourse import library_config

    assert nw % cg == 0
    nch = nw // cg
    n_lo = nw * t_lo * P
    n_hi = nw * t_hi * P

    nc = bacc.Bacc(
        "TRN2", target_bir_lowering=False, debug=False, num_swdge_queues=2,
        dynamic_dma_scratch_size=dma_scratch,
    )
    f32 = mybir.dt.float32
    tbl_lo = nc.dram_tensor("tbl_lo", [lo_rows, D], f32, kind="ExternalInput")
    tbl_hi = nc.dram_tensor("tbl_hi", [hi_rows, D], f32, kind="ExternalInput")
    idx_lo = nc.dram_tensor(
        "idx_lo", [P, n_lo // 16], mybir.dt.int16, kind="ExternalInput"
    )
    idx_hi = nc.dram_tensor(
        "idx_hi", [P, n_hi // 16], mybir.dt.int16, kind="ExternalInput"
    )
    dloc_lo = nc.dram_tensor("dloc_lo", [P, nw * t_lo], f32, kind="ExternalInput")
    dloc_hi = nc.dram_tensor("dloc_hi", [P, nw * t_hi], f32, kind="ExternalInput")
    wmat = nc.dram_tensor("wmat", [D, D], f32, kind="ExternalInput")
    iota = nc.dram_tensor("iota", [P, P], f32, kind="ExternalInput")
    out = nc.dram_tensor("out", [nw * P, D], f32, kind="ExternalOutput")

    with tile.TileContext(nc) as tc:
        nc.gpsimd.load_library(library_config.mlp)
        with (
            tc.tile_pool(name="const", bufs=1) as cpool,
            tc.tile_pool(name="msga", bufs=2) as apool,
            tc.tile_pool(name="msgb", bufs=2) as bpool,
            tc.tile_pool(name="sel", bufs=8) as spool,
            tc.tile_pool(name="agg", bufs=4) as gpool,
            tc.tile_pool(name="outp", bufs=4) as opool,
            tc.tile_pool(name="psw", bufs=4, space="PSUM") as pwpool,
            tc.tile_pool(name="pso", bufs=2, space="PSUM") as popool,
        ):
            w_sb = cpool.tile([D, D], f32, tag="w")
            nc.sync.dma_start(out=w_sb[:], in_=wmat.ap())
            iota_sb = cpool.tile([P, P], f32, tag="iota")
            nc.sync.dma_start(out=iota_sb[:], in_=iota.ap())
            il_sb = cpool.tile([P, n_lo // 16], mybir.dt.int16, tag="il")
            nc.sync.dma_start(out=il_sb[:], in_=idx_lo.ap())
            ih_sb = cpool.tile([P, n_hi // 16], mybir.dt.int16, tag="ih")
            nc.sync.dma_start(out=ih_sb[:], in_=idx_hi.ap())
            dl_sb = cpool.tile([P, nw * t_lo], f32, tag="dl")
            nc.sync.dma_start(out=dl_sb[:], in_=dloc_lo.ap())
            dh_sb = cpool.tile([P, nw * t_hi], f32, tag="dh")
            nc.sync.dma_start(out=dh_sb[:], in_=dloc_hi.ap())

            def body():
              for ch in range(nch):
                a_tile = apool.tile([P, cg * t_lo, D], f32, tag="msga")
                b_tile = bpool.tile([P, cg * t_hi, D], f32, tag="msgb")
                if parts in ("all", "gather"):
                    _emit_gathers(ch, a_tile, b_tile)
                if parts in ("all", "compute"):
                    _emit_compute(ch, a_tile, b_tile)

            def _emit_gathers(ch, a_tile, b_tile):
                nc.gpsimd.dma_gather(
                    a_tile[:],
                    tbl_lo.ap(),
                    il_sb[:, ch * cg * t_lo * 8 : (ch + 1) * cg * t_lo * 8],
                    cg * t_lo * P,
                    cg * t_lo * P,
                    D,
                    queue_num=0,
                    single_packet=False,
                )
                nc.gpsimd.dma_gather(
                    b_tile[:],
                    tbl_hi.ap(),
                    ih_sb[:, ch * cg * t_hi * 8 : (ch + 1) * cg * t_hi * 8],
                    cg * t_hi * P,
                    cg * t_hi * P,
                    D,
                    queue_num=1,
                    single_packet=False,
                )

            def _emit_compute(ch, a_tile, b_tile):
                for wi in range(cg):
                    w = ch * cg + wi
                    psw = pwpool.tile([P, P], f32, tag="psw")
                    nmm = t_lo + t_hi
                    k = 0
                    for t in range(t_lo):
                        sel = spool.tile([P, P], f32, tag="sel")
                        nc.vector.tensor_scalar(
                            sel[:],
                            iota_sb[:],
                            dl_sb[:, w * t_lo + t : w * t_lo + t + 1],
                            None,
                            mybir.AluOpType.is_equal,
                        )
                        nc.tensor.matmul(
                            psw[:],
                            a_tile[:, wi * t_lo + t, :],
                            sel[:],
                            start=(k == 0),
                            stop=(k == nmm - 1),
                        )
                        k += 1
                    for t in range(t_hi):
                        sel = spool.tile([P, P], f32, tag="sel")
                        nc.vector.tensor_scalar(
                            sel[:],
                            iota_sb[:],
                            dh_sb[:, w * t_hi + t : w * t_hi + t + 1],
                            None,
                            mybir.AluOpType.is_equal,
                        )
                        nc.tensor.matmul(
                            psw[:],
                            b_tile[:, wi * t_hi + t, :],
                            sel[:],
                            start=(k == 0),
                            stop=(k == nmm - 1),
                        )
                        k += 1
                    # psw is aggT for this window: [dim, dst_local]
                    agg_t = gpool.tile([P, P], f32, tag="agg")
                    nc.scalar.copy(agg_t[:], psw[:])
                    pso = popool.tile([P, P], f32, tag="pso")
                    nc.tensor.matmul(
                        pso[:], agg_t[:], w_sb[:], start=True, stop=True
                    )
                    o_sb = opool.tile([P, D], f32, tag="out")
                    nc.scalar.activation(
                        o_sb[:], pso[:], mybir.ActivationFunctionType.Relu
                    )
                    nc.sync.dma_start(
                        out=out.ap()[w * P : (w + 1) * P, :], in_=o_sb[:]
                    )

            if bench_reps == 1:
                body()
            else:
                # benchmarking only: repeat the whole body on-device so one
                # PJRT dispatch amortizes its ~90ms overhead over many runs
                with tc.For_i(0, bench_reps, 1):
                    body()

    nc.compile()
    _NC_CACHE[key] = nc
    return nc


def _grid(bucket, mask, order_vals_idx, order_vals_dloc, t, nw=NW, n_cores=N_CORES):
    """Pack one src-half's edges into the fixed per-core slot grid.

    bucket: per-edge (core * nw + window) id, mask: this half's edges.
    Returns idx16 [n_cores, 128, n/16] (int16, wrapped+replicated) and
    dloc [n_cores, 128, nw*t] (f32, -1 for pad slots).
    """
    nb = n_cores * nw
    b = bucket[mask]
    order = np.argsort(b, kind="stable")
    b_sorted = b[order]
    cnts = np.bincount(b_sorted, minlength=nb)
    starts = np.concatenate([[0], np.cumsum(cnts)[:-1]])
    rank = np.arange(len(b_sorted)) - starts[b_sorted]
    spb = t * P  # slots per bucket (window)
    n = nw * spb
    flat_idx = np.zeros((n_cores, n), dtype=np.int16)
    flat_dloc = np.full((n_cores, n), -1.0, dtype=np.float32)
    c = b_sorted // nw
    wloc = b_sorted % nw
    pos = wloc * spb + rank
    flat_idx[c, pos] = order_vals_idx[mask][order]
    flat_dloc[c, pos] = order_vals_dloc[mask][order]
    idx16 = flat_idx.reshape(n_cores, n // 16, 16).transpose(0, 2, 1)
    idx16 = np.ascontiguousarray(np.tile(idx16, (1, 8, 1)))
    dl = np.ascontiguousarray(flat_dloc.reshape(n_cores, nw * t, P).transpose(0, 2, 1))
    return idx16, dl


def kernel(x, edge_index, W):
    x = np.asarray(x, dtype=np.float32)
    edge_index = np.asarray(edge_index)
    W = np.asarray(W, dtype=np.float32)
    assert x.shape == (N_NODES, D) and W.shape == (D, D)

    src = edge_index[0].astype(np.int64)
    dst = edge_index[1].astype(np.int64)

    core = dst // DPC
    dl_all = dst - core * DPC
    w_all = dl_all >> 7
    dloc_all = (dl_all & 127).astype(np.float32)
    bucket = core * NW + w_all
    is_hi = src >= LO_MAX

    nb = N_CORES * NW
    cnt_lo = np.bincount(bucket[~is_hi], minlength=nb)
    cnt_hi = np.bincount(bucket[is_hi], minlength=nb)
    t_lo = max(1, int(np.ceil(cnt_lo.max() / P)))
    t_hi = max(1, int(np.ceil(cnt_hi.max() / P)))

    idx_val_lo = (src + 1).astype(np.int16, casting="unsafe")
    idx_val_hi = (src - LO_MAX + 1).astype(np.int16, casting="unsafe")
    idx16_lo, dloc_lo = _grid(bucket, ~is_hi, idx_val_lo, dloc_all, t_lo)
    idx16_hi, dloc_hi = _grid(bucket, is_hi, idx_val_hi, dloc_all, t_hi)

    tbl_lo = np.zeros((LO_ROWS, D), np.float32)
    tbl_lo[1:] = x[:LO_MAX]
    tbl_hi = np.zeros((HI_ROWS, D), np.float32)
    tbl_hi[1:] = x[LO_MAX:]
    iota = np.tile(np.arange(P, dtype=np.float32), (P, 1))
    iota = np.ascontiguousarray(iota)

    nc = _build_nc(t_lo, t_hi)

    in_maps = []
    for c in range(N_CORES):
        in_maps.append(
            {
                "tbl_lo": tbl_lo,
                "tbl_hi": tbl_hi,
                "idx_lo": idx16_lo[c],
                "idx_hi": idx16_hi[c],
                "dloc_lo": dloc_lo[c],
                "dloc_hi": dloc_hi[c],
                "wmat": W,
                "iota": iota,
            }
        )

    from concourse.bass_utils import run_bass_kernel_spmd

    res = run_bass_kernel_spmd(nc, in_maps, core_ids=list(range(N_CORES)))
    # stashed so a test harness can re-run / re-time this invocation
    global _LAST_RUN, _LAST_CAPS
    _LAST_RUN = (nc, in_maps)
    _LAST_CAPS = (t_lo, t_hi)
    outs = [res.results[c]["out"] for c in range(N_CORES)]
    full = np.concatenate(outs, axis=0)[:N_NODES]
    return np.ascontiguousarray(full)


_LAST_RUN = None
_LAST_CAPS = None



# revision 3
# speedup vs baseline: 1.2498x; 1.2498x over previous
"""GNN message passing on 8 Trainium2 NeuronCores.

Reference computation:
    h = x @ W                       # [N, D]
    msg = h[src]                    # [E, D]
    out = relu(segment_sum(msg, dst, N))

Key identity used here: segment_sum(x[src] @ W) == segment_sum(x[src]) @ W,
so we aggregate raw x rows and apply the small 128x128 matmul once per
output tile at the end.

Distribution: shard edges by DESTINATION range (6272 nodes per core).
Outputs are disjoint -> no all-reduce. Each core:
  1. dma_gather's x rows for its edges (from a replicated node table in
     its local HBM; the table is split into "lo"/"hi" halves so gather
     indices fit in int16, with a zeros row at index 0 for padding),
  2. aggregates them into per-128-dst-window PSUM accumulators via
     TensorE matmuls against one-hot selection matrices built on-device
     with a single DVE compare (iota == dst_local) per 128-edge tile,
  3. applies the final @W matmul + relu per window and DMAs out.

Host-side packing sorts edges into a fixed (window, src-half) slot grid
so all 8 cores run one identical SPMD program.
"""

import numpy as np

P = 128
D = 128
N_NODES = 50000
N_CORES = 8
NW = 49                 # 128-dst windows per core
DPC = NW * P            # dsts per core = 6272
CG = 7                  # windows per gather chunk (NW % CG == 0)
LO_MAX = 32767          # src < LO_MAX -> lo table (idx = src+1 <= 32767)
LO_ROWS = LO_MAX + 1
HI_ROWS = N_NODES - LO_MAX + 1

_NC_CACHE = {}


def _build_nc(t_lo, t_hi, nw=NW, cg=CG, lo_rows=LO_ROWS, hi_rows=HI_ROWS,
              bench_reps=1, parts="all", dma_scratch=16384):
    key = (t_lo, t_hi, nw, cg, lo_rows, hi_rows, bench_reps, parts, dma_scratch)
    if key in _NC_CACHE:
        return _NC_CACHE[key]

    import concourse.bacc as bacc
    import concourse.mybir as mybir
    import concourse.tile as tile
    from concourse import library_config

    assert nw % cg == 0
    nch = nw // cg
    n_lo = nw * t_lo * P
    n_hi = nw * t_hi * P

    nc = bacc.Bacc(
        "TRN2", target_bir_lowering=False, debug=False, num_swdge_queues=2,
        dynamic_dma_scratch_size=dma_scratch,
    )
    f32 = mybir.dt.float32
    tbl_lo = nc.dram_tensor("tbl_lo", [lo_rows, D], f32, kind="ExternalInput")
    tbl_hi = nc.dram_tensor("tbl_hi", [hi_rows, D], f32, kind="ExternalInput")
    idx_lo = nc.dram_tensor(
        "idx_lo", [P, n_lo // 16], mybir.dt.int16, kind="ExternalInput"
    )
    idx_hi = nc.dram_tensor(
        "idx_hi", [P, n_hi // 16], mybir.dt.int16, kind="ExternalInput"
    )
    dloc_lo = nc.dram_tensor("dloc_lo", [P, nw * t_lo], f32, kind="ExternalInput")
    dloc_hi = nc.dram_tensor("dloc_hi", [P, nw * t_hi], f32, kind="ExternalInput")
    wmat = nc.dram_tensor("wmat", [D, D], f32, kind="ExternalInput")
    iota = nc.dram_tensor("iota", [P, P], f32, kind="ExternalInput")
    out = nc.dram_tensor("out", [nw * P, D], f32, kind="ExternalOutput")

    with tile.TileContext(nc) as tc:
        nc.gpsimd.load_library(library_config.mlp)
        with (
            tc.tile_pool(name="const", bufs=1) as cpool,
            tc.tile_pool(name="msga", bufs=2) as apool,
            tc.tile_pool(name="msgb", bufs=2) as bpool,
            tc.tile_pool(name="sel", bufs=8) as spool,
            tc.tile_pool(name="agg", bufs=4) as gpool,
            tc.tile_pool(name="outp", bufs=4) as opool,
            tc.tile_pool(name="psw", bufs=4, space="PSUM") as pwpool,
            tc.tile_pool(name="pso", bufs=2, space="PSUM") as popool,
        ):
            w_sb = cpool.tile([D, D], f32, tag="w")
            nc.sync.dma_start(out=w_sb[:], in_=wmat.ap())
            iota_sb = cpool.tile([P, P], f32, tag="iota")
            nc.sync.dma_start(out=iota_sb[:], in_=iota.ap())
            il_sb = cpool.tile([P, n_lo // 16], mybir.dt.int16, tag="il")
            nc.sync.dma_start(out=il_sb[:], in_=idx_lo.ap())
            ih_sb = cpool.tile([P, n_hi // 16], mybir.dt.int16, tag="ih")
            nc.sync.dma_start(out=ih_sb[:], in_=idx_hi.ap())
            dl_sb = cpool.tile([P, nw * t_lo], f32, tag="dl")
            nc.sync.dma_start(out=dl_sb[:], in_=dloc_lo.ap())
            dh_sb = cpool.tile([P, nw * t_hi], f32, tag="dh")
            nc.sync.dma_start(out=dh_sb[:], in_=dloc_hi.ap())

            def body():
              for ch in range(nch):
                a_tile = apool.tile([P, cg * t_lo, D], f32, tag="msga")
                b_tile = bpool.tile([P, cg * t_hi, D], f32, tag="msgb")
                if parts in ("all", "gather"):
                    _emit_gathers(ch, a_tile, b_tile)
                elif parts == "compute":
                    # timing-isolation mode: tiny gather just to mark the
                    # tiles written so the tile allocator accepts the reads
                    nc.gpsimd.dma_gather(
                        a_tile[:, :1, :], tbl_lo.ap(), il_sb[:, :8],
                        P, P, D, queue_num=0, single_packet=False,
                    )
                    nc.gpsimd.dma_gather(
                        b_tile[:, :1, :], tbl_hi.ap(), ih_sb[:, :8],
                        P, P, D, queue_num=1, single_packet=False,
                    )
                if parts in ("all", "compute"):
                    _emit_compute(ch, a_tile, b_tile)

            def _emit_gathers(ch, a_tile, b_tile):
                nc.gpsimd.dma_gather(
                    a_tile[:],
                    tbl_lo.ap(),
                    il_sb[:, ch * cg * t_lo * 8 : (ch + 1) * cg * t_lo * 8],
                    cg * t_lo * P,
                    cg * t_lo * P,
                    D,
                    queue_num=0,
                    single_packet=False,
                )
                nc.gpsimd.dma_gather(
                    b_tile[:],
                    tbl_hi.ap(),
                    ih_sb[:, ch * cg * t_hi * 8 : (ch + 1) * cg * t_hi * 8],
                    cg * t_hi * P,
                    cg * t_hi * P,
                    D,
                    queue_num=1,
                    single_packet=False,
                )

            def _emit_compute(ch, a_tile, b_tile):
                for wi in range(cg):
                    w = ch * cg + wi
                    psw = pwpool.tile([P, P], f32, tag="psw")
                    nmm = t_lo + t_hi
                    k = 0
                    for t in range(t_lo):
                        sel = spool.tile([P, P], f32, tag="sel")
                        nc.vector.tensor_scalar(
                            sel[:],
                            iota_sb[:],
                            dl_sb[:, w * t_lo + t : w * t_lo + t + 1],
                            None,
                            mybir.AluOpType.is_equal,
                        )
                        nc.tensor.matmul(
                            psw[:],
                            a_tile[:, wi * t_lo + t, :],
                            sel[:],
                            start=(k == 0),
                            stop=(k == nmm - 1),
                        )
                        k += 1
                    for t in range(t_hi):
                        sel = spool.tile([P, P], f32, tag="sel")
                        nc.vector.tensor_scalar(
                            sel[:],
                            iota_sb[:],
                            dh_sb[:, w * t_hi + t : w * t_hi + t + 1],
                            None,
                            mybir.AluOpType.is_equal,
                        )
                        nc.tensor.matmul(
                            psw[:],
                            b_tile[:, wi * t_hi + t, :],
                            sel[:],
                            start=(k == 0),
                            stop=(k == nmm - 1),
                        )
                        k += 1
                    # psw is aggT for this window: [dim, dst_local]
                    agg_t = gpool.tile([P, P], f32, tag="agg")
                    nc.scalar.copy(agg_t[:], psw[:])
                    pso = popool.tile([P, P], f32, tag="pso")
                    nc.tensor.matmul(
                        pso[:], agg_t[:], w_sb[:], start=True, stop=True
                    )
                    o_sb = opool.tile([P, D], f32, tag="out")
                    nc.scalar.activation(
                        o_sb[:], pso[:], mybir.ActivationFunctionType.Relu
                    )
                    nc.sync.dma_start(
                        out=out.ap()[w * P : (w + 1) * P, :], in_=o_sb[:]
                    )

            if bench_reps == 1:
                body()
            else:
                # benchmarking only: repeat the whole body on-device so one
                # PJRT dispatch amortizes its ~90ms overhead over many runs
                with tc.For_i(0, bench_reps, 1):
                    body()

    nc.compile()
    _NC_CACHE[key] = nc
    return nc


def _grid(bucket, mask, order_vals_idx, order_vals_dloc, t, nw=NW, n_cores=N_CORES):
    """Pack one src-half's edges into the fixed per-core slot grid.

    bucket: per-edge (core * nw + window) id, mask: this half's edges.
    Returns idx16 [n_cores, 128, n/16] (int16, wrapped+replicated) and
    dloc [n_cores, 128, nw*t] (f32, -1 for pad slots).
    """
    nb = n_cores * nw
    b = bucket[mask]
    order = np.argsort(b, kind="stable")
    b_sorted = b[order]
    cnts = np.bincount(b_sorted, minlength=nb)
    starts = np.concatenate([[0], np.cumsum(cnts)[:-1]])
    rank = np.arange(len(b_sorted)) - starts[b_sorted]
    spb = t * P  # slots per bucket (window)
    n = nw * spb
    flat_idx = np.zeros((n_cores, n), dtype=np.int16)
    flat_dloc = np.full((n_cores, n), -1.0, dtype=np.float32)
    c = b_sorted // nw
    wloc = b_sorted % nw
    pos = wloc * spb + rank
    flat_idx[c, pos] = order_vals_idx[mask][order]
    flat_dloc[c, pos] = order_vals_dloc[mask][order]
    idx16 = flat_idx.reshape(n_cores, n // 16, 16).transpose(0, 2, 1)
    idx16 = np.ascontiguousarray(np.tile(idx16, (1, 8, 1)))
    dl = np.ascontiguousarray(flat_dloc.reshape(n_cores, nw * t, P).transpose(0, 2, 1))
    return idx16, dl


def _prepare(x, edge_index, W):
    """Host-side packing: returns (t_lo, t_hi, in_maps)."""
    x = np.asarray(x, dtype=np.float32)
    edge_index = np.asarray(edge_index)
    W = np.asarray(W, dtype=np.float32)
    assert x.shape == (N_NODES, D) and W.shape == (D, D)

    src = edge_index[0].astype(np.int64)
    dst = edge_index[1].astype(np.int64)

    core = dst // DPC
    dl_all = dst - core * DPC
    w_all = dl_all >> 7
    dloc_all = (dl_all & 127).astype(np.float32)
    bucket = core * NW + w_all
    is_hi = src >= LO_MAX

    nb = N_CORES * NW
    cnt_lo = np.bincount(bucket[~is_hi], minlength=nb)
    cnt_hi = np.bincount(bucket[is_hi], minlength=nb)
    t_lo = max(1, int(np.ceil(cnt_lo.max() / P)))
    t_hi = max(1, int(np.ceil(cnt_hi.max() / P)))

    idx_val_lo = (src + 1).astype(np.int16, casting="unsafe")
    idx_val_hi = (src - LO_MAX + 1).astype(np.int16, casting="unsafe")
    idx16_lo, dloc_lo = _grid(bucket, ~is_hi, idx_val_lo, dloc_all, t_lo)
    idx16_hi, dloc_hi = _grid(bucket, is_hi, idx_val_hi, dloc_all, t_hi)

    tbl_lo = np.zeros((LO_ROWS, D), np.float32)
    tbl_lo[1:] = x[:LO_MAX]
    tbl_hi = np.zeros((HI_ROWS, D), np.float32)
    tbl_hi[1:] = x[LO_MAX:]
    iota = np.tile(np.arange(P, dtype=np.float32), (P, 1))
    iota = np.ascontiguousarray(iota)

    in_maps = []
    for c in range(N_CORES):
        in_maps.append(
            {
                "tbl_lo": tbl_lo,
                "tbl_hi": tbl_hi,
                "idx_lo": idx16_lo[c],
                "idx_hi": idx16_hi[c],
                "dloc_lo": dloc_lo[c],
                "dloc_hi": dloc_hi[c],
                "wmat": W,
                "iota": iota,
            }
        )
    return t_lo, t_hi, in_maps


def kernel(x, edge_index, W):
    t_lo, t_hi, in_maps = _prepare(x, edge_index, W)
    nc = _build_nc(t_lo, t_hi)

    from concourse.bass_utils import run_bass_kernel_spmd

    res = run_bass_kernel_spmd(nc, in_maps, core_ids=list(range(N_CORES)))
    # stashed so a test harness can re-run / re-time this invocation
    global _LAST_RUN, _LAST_CAPS
    _LAST_RUN = (nc, in_maps)
    _LAST_CAPS = (t_lo, t_hi)
    outs = [res.results[c]["out"] for c in range(N_CORES)]
    full = np.concatenate(outs, axis=0)[:N_NODES]
    return np.ascontiguousarray(full)


_LAST_RUN = None
_LAST_CAPS = None

